# revision 54
# baseline (speedup 1.0000x reference)
"""GQA attention (B=1, T=2048, D=2048, 32 q heads / 8 kv heads, DH=64, RoPE,
causal) on 8 Trainium2 NeuronCores, tensor-parallel over heads.

Per core: 1 kv head + its 4 q heads (2 pairs). Kernel computes, per core,
partial = (softmax(rope(Q) rope(K)^T / 8) V) @ Wo_shard ; host sums partials.

Layout strategy (all on-chip matmuls contract over the partition dim):
  - host supplies x^T [D, T] in bf16; QKV projections use Wq/Wkv as lhsT,
    x^T as rhs (bf16 matmul = 1 cycle/row, fp32 PSUM accumulate)
  - scores are built transposed: S^T[ts, tq] = K^T_chunk.T @ Q^T  (no P
    transposes needed for the AV matmul); diagonal blocks use reduced
    column ranges (only tq >= ts_block_start) to skip fully-masked work
  - V' = [V | 1] column gives the softmax denominator for free in row 64 of
    the AV accumulator; V^T is produced by per-block PE transposes
  - RoPE is interleaved per-tq-block into the QKV stage (DVE hides under PE)
  - q-head pairs run concurrently in the PE array via row tile_position
  - AV matmuls lag S/exp by one iteration and group finalizes are deferred,
    so the ACT exp stream never stalls at group seams; elementwise work is
    spread across DVE/Pool/ACT; DMAs are spread across the SP/ACT/Pool DGE
    queues (GPSIMD never touches PSUM - hardware restriction)
"""

import numpy as np
from contextlib import ExitStack

import ml_dtypes

from concourse import bacc
import concourse.mybir as mybir
import concourse.tile as tile
from concourse.bass_utils import run_bass_kernel_spmd
from concourse.masks import make_identity

B, T, D = 1, 2048, 2048
NH, NKV, DH = 32, 8, 64
NCORES = 8
HPC = NH // NCORES      # 4 q heads per core
PAIRS = HPC // 2        # 2
TB = 512                # tq block (one psum bank of fp32)
NTB = T // TB           # 4
NKT = D // 128          # 16 contraction tiles
NTS = T // 128          # 16 ts blocks
SCALE = 1.0 / float(np.sqrt(DH))

F32 = mybir.dt.float32
BF16 = mybir.dt.bfloat16
EXP = mybir.ActivationFunctionType.Exp
BF16NP = ml_dtypes.bfloat16

_CACHE = {}


def build_nc():
    nc = bacc.Bacc(None, target_bir_lowering=False)

    x8 = nc.declare_dram_parameter("x8", [64, NTB, NKT, 2, TB], F8, isOutput=False)
    wq = nc.declare_dram_parameter("wq", [64, 2, PAIRS, NKT, 128], F8, isOutput=False)
    wkv = nc.declare_dram_parameter("wkv", [64, 2, NKT, 128], F8, isOutput=False)
    wo = nc.declare_dram_parameter("wo", [128, 2, T], BF16, isOutput=False)
    tcc = nc.declare_dram_parameter("tcc", [128, T], BF16, isOutput=False)
    tss = nc.declare_dram_parameter("tss", [128, T], BF16, isOutput=False)
    bm = nc.declare_dram_parameter("bm", [128, 128], BF16, isOutput=False)
    out = nc.declare_dram_parameter("out", [T, D], BF16, isOutput=True)

    with tile.TileContext(nc) as tc, ExitStack() as top:
        per = top.enter_context(tc.tile_pool(name="persist", bufs=1))

        wq_sb = per.tile([64, 2, PAIRS, NKT, 128], F8, tag="wq")
        wkv_sb = per.tile([64, 2, NKT, 128], F8, tag="wkv")
        wo_sb = per.tile([128, 2, T], BF16, tag="wo")
        tcc_sb = per.tile([128, T], BF16, tag="tcc")
        tss_sb = per.tile([128, T], BF16, tag="tss")
        bm_sb = per.tile([128, 128], BF16, tag="bm")
        ones = per.tile([65, 64], BF16, tag="ones")
        ident = per.tile([128, 128], BF16, tag="ident")

        # qkvt rows: [q pair0 | q pair1 | K(0:64);V(64:128)] as [128, 3, T]
        qkvt = per.tile([128, 3, T], BF16, tag="qkvt")
        qrf8 = per.tile([128, PAIRS, T], F8, tag="qrf8")
        kkf8 = per.tile([64, T], F8, tag="kkf8")
        kk8 = per.tile([32, 2, T], F8, tag="kk8")
        qr8 = per.tile([32, 4, PAIRS, T], F8, tag="qr8")
        v_sb = per.tile([128, NTS, 65], BF16, tag="v")
        ytall = per.tile([128, PAIRS, T], BF16, tag="yt")

        nc.scalar.dma_start(out=wq_sb[:, :, :, 0:4], in_=wq[:, :, :, 0:4])
        nc.scalar.dma_start(out=wkv_sb[:, :, 0:4], in_=wkv[:, :, 0:4])
        nc.vector.memset(ones, 1.0)
        make_identity(nc, ident)
        onesv = per.tile([128, NTS, 1], BF16, tag="onesv")
        nc.vector.memset(onesv, 1.0)
        nc.vector.tensor_copy(v_sb[:, :, 64:65], onesv)

        # ---- Stage A+B: QKV projections + RoPE, per tq block of 512
        with (
            tc.tile_pool(name="psA", bufs=6, space="PSUM") as psA,
            tc.tile_pool(name="psV", bufs=2, space="PSUM") as psV,
            tc.tile_pool(name="xs", bufs=3) as xs,
        ):
            xsrc = x8[:]
            xts = [xs.tile([64, NKT, 2, TB], F8, tag="x", name=f"x{tb}")
                   for tb in range(NTB)]

            def load_x(tb):
                nc.sync.dma_start(out=xts[tb], in_=xsrc[:, tb])

            # issue order tuned so PE never starves and rope DMAs find
            # idle DMA engines: x loads run 2 blocks ahead of compute.
            nc.sync.dma_start(out=xts[0][:, 0:4], in_=xsrc[:, 0, 0:4])
            nc.sync.dma_start(out=xts[0][:, 4:10], in_=xsrc[:, 0, 4:10])
            nc.scalar.dma_start(out=wq_sb[:, :, :, 4:16], in_=wq[:, :, :, 4:16])
            nc.scalar.dma_start(out=wkv_sb[:, :, 4:16], in_=wkv[:, :, 4:16])
            nc.sync.dma_start(out=xts[0][:, 10:16], in_=xsrc[:, 0, 10:16])
            nc.sync.dma_start(out=xts[1][:, 0:8], in_=xsrc[:, 1, 0:8])
            nc.scalar.dma_start(out=tcc_sb[:, 0:TB], in_=tcc[:, 0:TB])
            nc.scalar.dma_start(out=tss_sb[:, 0:TB], in_=tss[:, 0:TB])
            nc.sync.dma_start(out=xts[1][:, 8:16], in_=xsrc[:, 1, 8:16])

            for tb in range(NTB):
                cs = slice(tb * TB, (tb + 1) * TB)
                xt = xts[tb]
                if tb == 0:
                    load_x(2)
                    nc.scalar.dma_start(out=tcc_sb[:, TB:], in_=tcc[:, TB:])
                    nc.scalar.dma_start(out=tss_sb[:, TB:], in_=tss[:, TB:])
                    nc.scalar.dma_start(out=bm_sb, in_=bm[:])
                elif tb == 1:
                    load_x(3)
                    nc.scalar.dma_start(out=wo_sb, in_=wo[:])
                accs = [psA.tile([128, TB], F32, tag="acc", name=f"acc{tb}_{i}")
                        for i in range(3)]
                for kt in range(NKT):
                    st, sp = kt == 0, kt == NKT - 1
                    nc.tensor.matmul(accs[0], wq_sb[:, :, 0, kt], xt[:, kt],
                                     start=st, stop=sp, perf_mode=DROW)
                    nc.tensor.matmul(accs[1], wq_sb[:, :, 1, kt], xt[:, kt],
                                     start=st, stop=sp, perf_mode=DROW)
                    nc.tensor.matmul(accs[2], wkv_sb[:, :, kt], xt[:, kt],
                                     start=st, stop=sp, perf_mode=DROW)
                if tb == NTB - 1:
                    nc.vector.tensor_copy(qkvt[:, 0, cs], accs[0])
                    nc.vector.tensor_copy(qkvt[:, 1, cs], accs[1])
                else:
                    nc.scalar.copy(qkvt[:, 0, cs], accs[0])
                    nc.scalar.copy(qkvt[:, 1, cs], accs[1])
                nc.scalar.copy(qkvt[:, 2, cs], accs[2])

                # RoPE on this block: rotate-half via partition-swap DMA
                # (one DMA covers q pair0, q pair1 and K), then
                # qr = q*cc + rot(q)*ss with signs folded into tss.
                rot = per.tile([128, 3, TB], BF16, tag=f"rot{tb}", name=f"rot{tb}")
                for b, eng in ((0, nc.gpsimd), (64, nc.sync)):
                    eng.dma_start(out=rot[b:b + 32], in_=qkvt[b + 32:b + 64, :, cs])
                    eng.dma_start(out=rot[b + 32:b + 64], in_=qkvt[b:b + 32, :, cs])
                for g in range(PAIRS):
                    p1 = per.tile([128, TB], BF16, tag=f"p1_{tb}_{g}", name=f"p1{tb}_{g}")
                    nc.vector.tensor_mul(p1, qkvt[:, g, cs], tcc_sb[:, cs])
                    nc.vector.tensor_mul(rot[:, g], rot[:, g], tss_sb[:, cs])
                    nc.vector.tensor_add(qrf8[:, g, cs], p1, rot[:, g])
                p1 = per.tile([128, TB], BF16, tag=f"p1k{tb}", name=f"p1k{tb}")
                nc.vector.tensor_mul(p1[0:64], qkvt[0:64, 2, cs], tcc_sb[0:64, cs])
                nc.vector.tensor_mul(rot[0:64, 2], rot[0:64, 2], tss_sb[0:64, cs])
                nc.vector.tensor_add(kkf8[:, cs], p1[0:64], rot[0:64, 2])
                # partition folds for fp8 DoubleRow scores (plain slices so
                # the dependency tracker sees them; no cast -> hwdge ok):
                # kk8[p,i,t] = K[t, 32i+p]; qr8[p,2h+i,g,t] = Q_{g,h}[t, 32i+p]
                for j, eng in ((0, nc.sync), (1, nc.scalar),
                               (2, nc.sync), (3, nc.gpsimd)):
                    eng.dma_start(out=qr8[:, j, :, cs],
                                  in_=qrf8[32 * j:32 * (j + 1), :, cs])
                for i, eng in ((0, nc.sync), (1, nc.scalar)):
                    eng.dma_start(out=kk8[:, i, cs],
                                  in_=kkf8[32 * i:32 * (i + 1), cs])
                # V^T for this block's 4 ts tiles via PE transposes
                for tt in range(4 * tb, 4 * tb + 4):
                    vp = psV.tile([128, 64], BF16, tag="vt", name=f"vt{tt}")
                    nc.tensor.transpose(
                        vp, qkvt[64:128, 2, tt * 128:(tt + 1) * 128],
                        ident[64:128, 64:128],
                    )
                    nc.vector.tensor_copy(v_sb[:, tt, 0:64], vp)

        # ---- Stage C: attention with stage-D (out-proj) matmuls interleaved
        # into the PE bubbles left by exp waits. Diagonal blocks only compute
        # columns tq >= ts_block_start (off), with a 128-wide triangular mask.
        with (
            tc.tile_pool(name="psS", bufs=2, space="PSUM") as psS,
            tc.tile_pool(name="psY", bufs=2, space="PSUM") as psY,
            tc.tile_pool(name="psO", bufs=2, space="PSUM") as psO,
            tc.tile_pool(name="esb", bufs=4) as esb,
            tc.tile_pool(name="rsb", bufs=2) as rsb,
            tc.tile_pool(name="osb", bufs=2) as osb,
        ):
            ready = []
            obs = {}
            drain = [False]

            def emit_d():
                tt, nb = ready.pop(0)
                tsl = slice(tt * 128, (tt + 1) * 128)
                nsl = slice(nb * TB, (nb + 1) * TB)
                po = psO.tile([128, TB], F32, tag="o", name=f"po{tt}_{nb}")
                nc.tensor.matmul(po, ytall[:, 0, tsl], wo_sb[:, 0, nsl],
                                 start=True, stop=False)
                nc.tensor.matmul(po, ytall[:, 1, tsl], wo_sb[:, 1, nsl],
                                 start=False, stop=True)
                if nb == 0:
                    obs[tt] = osb.tile([128, T], BF16, tag="ob", name=f"ob{tt}")
                if drain[0]:
                    if (tt * NTB + nb) % 2 == 0:
                        nc.scalar.copy(obs[tt][:, nsl], po)
                    else:
                        nc.vector.tensor_copy(obs[tt][:, nsl], po)
                    if tt == NTS - 1:
                        nc.sync.dma_start(out=out[tsl, nsl], in_=obs[tt][:, nsl])
                        if nb == NTB - 1:
                            obs.pop(tt)
                    elif nb == 1:
                        nc.sync.dma_start(out=out[tsl, 0:2 * TB],
                                          in_=obs[tt][:, 0:2 * TB])
                    elif nb == NTB - 1:
                        nc.sync.dma_start(out=out[tsl, 2 * TB:],
                                          in_=obs.pop(tt)[:, 2 * TB:])
                else:
                    nc.vector.tensor_copy(obs[tt][:, nsl], po)
                    if nb == NTB - 1:
                        nc.sync.dma_start(out=out[tsl, :], in_=obs.pop(tt))

            pending_fin = []
            pending_av = []

            def finalize(tb, g, ys):
                q0 = tb * TB
                qs = slice(q0, q0 + TB)
                for h in (1, 0):
                    yp = ys[:, h]
                    rec = rsb.tile([65, TB], BF16, tag="rec",
                                   name=f"rec{g}_{tb}_{h}")
                    with nc.allow_low_precision(reason="bf16 broadcast of 1/l"):
                        nc.vector.reciprocal(rec[64:65], yp[64:65, :])
                    # bc shares the psO ring (same tag/shape -> same banks)
                    bc = psO.tile([128, TB], F32, tag="o",
                                  name=f"bc{g}_{tb}_{h}")[0:64]
                    # fp32 (exact) broadcast of 1/l across partitions
                    nc.tensor.matmul(bc, ones[64:65], rec[64:65],
                                     start=True, stop=True)
                    if h == 0:
                        nc.vector.tensor_mul(ytall[0:64, g, qs], yp[0:64, :], bc)
                    else:
                        yn = rsb.tile([64, TB], BF16, tag=f"yn{g}",
                                      name=f"yn{g}_{tb}")
                        nc.vector.tensor_mul(yn, yp[0:64, :], bc)
                        nc.sync.dma_start(out=ytall[64:128, g, qs], in_=yn)
                del ys
                for tt in range(4 * tb, 4 * tb + 4):
                    for nb in range(NTB):
                        if g == 1:
                            ready.append((tt, nb))

            it = 0
            for tb in range(NTB):
                for g in range(PAIRS):
                    ya = psY.tile([65, TB], F32, tag="y", name=f"ya{g}_{tb}")
                    yb = psY.tile([65, TB], F32, tag="y", name=f"yb{g}_{tb}")
                    q0 = tb * TB
                    nts_here = 4 * tb + 4
                    for ts in range(nts_here):
                        ks = slice(ts * 128, (ts + 1) * 128)
                        o = ts - 4 * tb
                        off = max(0, o) * 128
                        qs = slice(q0 + off, q0 + TB)
                        sp = psS.tile([128, 2, TB], F32, tag="s",
                                      name=f"s{g}_{tb}_{ts}")
                        nc.tensor.matmul(
                            sp[:, 0, off:TB], kk8[:, :, ks], qr8[:, 0:2, g, qs],
                            start=True, stop=True, perf_mode=DROW,
                        )
                        nc.tensor.matmul(
                            sp[:, 1, off:TB], kk8[:, :, ks], qr8[:, 2:4, g, qs],
                            start=True, stop=True, perf_mode=DROW,
                        )
                        e = esb.tile([128, 2, TB], BF16, tag="e",
                                     name=f"e{g}_{tb}_{ts}")
                        nc.scalar.activation(e[:, :, off:TB], sp[:, :, off:TB],
                                             EXP, scale=SCALE)
                        if o >= 0:
                            nc.gpsimd.tensor_mul(e[:, 0, off:off + 128],
                                                 e[:, 0, off:off + 128], bm_sb)
                            nc.gpsimd.tensor_mul(e[:, 1, off:off + 128],
                                                 e[:, 1, off:off + 128], bm_sb)
                        # one-iteration-lagged AV: at group boundaries the
                        # next group's S/exp issue before the previous
                        # group's last AV, keeping ACT busy through the seam
                        if len(pending_av) > 1:
                            pending_av.pop(0)()
                        # software-pipelined finalize of the previous group
                        if ts == 1 and pending_fin:
                            finalize(*pending_fin.pop(0))

                        def make_av(tb=tb, g=g, ya=ya, yb=yb, e=e, off=off,
                                    ts=ts, st=ts == 0, last=ts == nts_here - 1):
                            def av():
                                nc.tensor.matmul(ya[:, off:TB], v_sb[:, ts],
                                                 e[:, 0, off:TB],
                                                 start=st, stop=last)
                                nc.tensor.matmul(yb[:, off:TB], v_sb[:, ts],
                                                 e[:, 1, off:TB],
                                                 start=st, stop=last)
                                if last:
                                    ys = rsb.tile([65, 2, TB], F32, tag="ys",
                                                  name=f"ys{g}_{tb}")
                                    nc.vector.tensor_copy(ys[:, 0], ya)
                                    nc.scalar.copy(ys[:, 1], yb)
                                    pending_fin.append((tb, g, ys))
                            return av

                        pending_av.append(make_av())
                        if ready:
                            emit_d()
                        it += 1
            drain[0] = True
            while pending_av:
                pending_av.pop(0)()
            while pending_fin:
                finalize(*pending_fin.pop(0))
            while ready:
                emit_d()

    nc.compile()
    if not nc.is_finalized():
        nc.finalize()
    return nc


def _prep_inputs(x, rc, rs, Wq, Wk, Wv, Wo):
    xT = np.ascontiguousarray(np.asarray(x, np.float32).reshape(T, D).T)
    x8 = np.ascontiguousarray(
        xT.reshape(NKT, 2, 64, NTB, TB).transpose(2, 3, 0, 1, 4)).astype(F8NP)
    csT = np.ascontiguousarray(np.asarray(rc, np.float32).T)   # [32, T]
    snT = np.ascontiguousarray(np.asarray(rs, np.float32).T)
    tcc = np.ascontiguousarray(np.concatenate([csT, csT, csT, csT], 0)).astype(BF16NP)
    tss = np.ascontiguousarray(np.concatenate([-snT, snT, -snT, snT], 0)).astype(BF16NP)
    u = np.arange(128)[None, :]
    p = np.arange(128)[:, None]
    bm = (u >= p).astype(BF16NP)

    Wq = np.asarray(Wq, np.float32)
    Wk = np.asarray(Wk, np.float32)
    Wv = np.asarray(Wv, np.float32)
    Wo = np.asarray(Wo, np.float32)
    in_maps = []
    for c in range(NCORES):
        wq_c = Wq[:, c * 256:(c + 1) * 256]               # [D, 256]
        wq_t = np.ascontiguousarray(
            wq_c.reshape(NKT, 2, 64, PAIRS, 128).transpose(2, 1, 3, 0, 4)
        ).astype(F8NP)                                     # [64,2,PAIRS,NKT,128]
        wkv_c = np.concatenate(
            [Wk[:, c * 64:(c + 1) * 64], Wv[:, c * 64:(c + 1) * 64]], 1
        )                                                  # [D, 128]
        wkv_t = np.ascontiguousarray(
            wkv_c.reshape(NKT, 2, 64, 128).transpose(2, 1, 0, 3)
        ).astype(F8NP)                                     # [64,2,NKT,128]
        wo_c = Wo[c * 256:(c + 1) * 256, :]                # [256, D]
        wo_t = np.ascontiguousarray(
            wo_c.reshape(2, 128, T).transpose(1, 0, 2)
        ).astype(BF16NP)
        in_maps.append(
            dict(x8=x8, wq=wq_t, wkv=wkv_t, wo=wo_t, tcc=tcc, tss=tss, bm=bm)
        )
    return in_maps


def kernel(x, rc, rs, Wq, Wk, Wv, Wo, _trace=False, _trace_kwargs=None):
    x = np.asarray(x, np.float32)
    if "nc" not in _CACHE:
        _CACHE["nc"] = build_nc()
    nc = _CACHE["nc"]
    in_maps = _prep_inputs(x, rc, rs, np.asarray(Wq), np.asarray(Wk),
                           np.asarray(Wv), np.asarray(Wo))
    kw = {}
    if _trace:
        kw = dict(trace=True, **(_trace_kwargs or {}))
    res = run_bass_kernel_spmd(nc, in_maps, list(range(NCORES)), **kw)
    parts = np.stack([np.asarray(res.results[i]["out"], np.float32)
                      for i in range(NCORES)])
    full = parts.sum(0, dtype=np.float64).astype(np.float32)
    kernel.last_result = res
    return full.reshape(B, T, D)
